# revision 9
# baseline (speedup 1.0000x reference)
"""Trainium2 Bass kernel for nn_DiffusionNCA_fft2 (8-core data-parallel).

Algorithm notes (validated in numpy to 2e-8 fp32 / 8e-5 bf16 vs reference):
  * The concat([dxn, conv0(dxn), conv1(dxn)]) @ fc0_w.T is folded into a
    single 49-tap stacked-matmul accumulation: for each tap k (7x7 window),
    C_k[hid, c] = fc0_w[:,35+c]*w1[c,k] + fc0_w[:,70+c]*w2[c,k] (+fc0_w[:,c]
    at the center tap).  fc0_out[:, pix] = sum_k C_k @ dxn[:, pix+delta_k].
  * 4 partition-blocks hold H-shifted copies of the reflect-padded
    normalized image (shifts -1,0,1,2 rows), so one matmul with a moving
    free-offset covers 4 taps at once -> 14 matmuls + ramp mm per 512-pixel
    tile, all accumulated in one PSUM bank.
  * The 3 extra channels (pos_x, pos_y, alive) are affine fields; their
    folded contribution is r*(p0 + p1*w + p2*h + D_border) + const vectors,
    where D is nonzero only in the 3-wide reflect border.  Interior handled
    by a tiny K=2 matmul over static (w, h) ramp rows; borders by small DVE
    adds on PSUM; p0-part goes into the per-tile activation bias.
  * GroupNorm stats: step-0 stats on host; step-1 stats fused into the
    residual pass (accum_out running sums + a Square pass).
"""

import math

import numpy as np
import ml_dtypes

import concourse.bass as bass
from concourse import bacc
import concourse.tile as tile
from concourse import mybir
from concourse import bass_isa
from concourse.bass_utils import run_bass_kernel_spmd

F32 = mybir.dt.float32
BF16 = mybir.dt.bfloat16
AF = mybir.ActivationFunctionType
OP = mybir.AluOpType

B, CH, HID, H, W = 8, 32, 128, 256, 256
STEPS, FIRE, EPS, C = 2, 0.5, 1e-5, 35
PAD = 3
HP = H + 2 * PAD          # 262
WP = W + 2 * PAD          # 262
NPIX = H * W              # 65536
NTILE = 128               # 512-pixel (2-row) tiles per step
TPX = NPIX // NTILE       # 512
NFLAT = HP * WP           # 68644
NSTAT = C * NPIX          # groupnorm element count
N_CORES = 8
FULL_TILES = (0, 1, 126, 127)   # tiles where D covers the whole tile


def _build_nc():
    nc = bacc.Bacc("TRN2", target_bir_lowering=False, debug=False)

    x_io = nc.dram_tensor("x_io", [CH, NPIX], F32, kind="ExternalInput")
    x_out = nc.dram_tensor("x_out", [CH, NPIX], F32, kind="ExternalOutput")
    cstk_io = nc.dram_tensor("cstk_io", [128, 14 * 128], BF16, kind="ExternalInput")
    fc1t_io = nc.dram_tensor("fc1t_io", [HID, CH], BF16, kind="ExternalInput")
    ramp_io = nc.dram_tensor("ramp_io", [2, TPX], BF16, kind="ExternalInput")
    p12_io = nc.dram_tensor("p12_io", [2, HID], F32, kind="ExternalInput")
    # vecs cols: 0 bias_base (fc0_b + convb + Kb), 1 p0, 2 Kg, 3 p2
    vecs_io = nc.dram_tensor("vecs_io", [HID, 4], F32, kind="ExternalInput")
    # gb cols: 0 gamma (g,c expanded), 1 beta
    gb_io = nc.dram_tensor("gb_io", [128, 2], F32, kind="ExternalInput")
    dcorr_io = nc.dram_tensor("dcorr_io", [HID, 4 * TPX + 124 * 12], BF16,
                              kind="ExternalInput")
    mask_io = nc.dram_tensor("mask_io", [STEPS, NPIX], BF16, kind="ExternalInput")
    # scal cols: 0 sum0_tot, 1 ssq0_tot, 2 pos_sum, 3 pos_ssq
    scal_io = nc.dram_tensor("scal_io", [1, 4], F32, kind="ExternalInput")

    with tile.TileContext(nc) as tc:
        with (
            tc.tile_pool(name="singles", bufs=1) as singles,
            tc.tile_pool(name="chunks", bufs=3) as chunks,
            tc.tile_pool(name="chunksb", bufs=3) as chunksb,
            tc.tile_pool(name="hpool", bufs=3) as hpool,
            tc.tile_pool(name="small", bufs=4) as small,
            tc.tile_pool(name="sc", bufs=2) as sc,
            tc.tile_pool(name="biasp", bufs=3) as biasp,
            tc.tile_pool(name="xio", bufs=4) as xio,
            tc.tile_pool(name="psA", bufs=2, space="PSUM") as psA,
            tc.tile_pool(name="psB", bufs=2, space="PSUM") as psB,
            tc.tile_pool(name="psJ", bufs=2, space="PSUM") as psJ,
        ):
            # ---- static loads -------------------------------------------------
            cstk = singles.tile([128, 14 * 128], BF16)
            nc.sync.dma_start(cstk[:], cstk_io[:])
            fc1t = singles.tile([HID, CH], BF16)
            nc.sync.dma_start(fc1t[:], fc1t_io[:])
            ramp = singles.tile([2, TPX], BF16)
            nc.sync.dma_start(ramp[:], ramp_io[:])
            p12 = singles.tile([2, HID], F32)
            nc.sync.dma_start(p12[:], p12_io[:])
            vecs = singles.tile([HID, 4], F32)
            nc.sync.dma_start(vecs[:], vecs_io[:])
            gb = singles.tile([128, 2], F32)
            nc.sync.dma_start(gb[:], gb_io[:])
            dcorr = singles.tile([HID, 4 * TPX + 124 * 12], BF16)
            nc.sync.dma_start(dcorr[:], dcorr_io[:])
            scal = singles.tile([1, 4], F32)
            nc.sync.dma_start(scal[:], scal_io[:])
            eps_sb = singles.tile([1, 1], F32)
            nc.vector.memset(eps_sb[:], EPS)

            dxn3 = singles.tile([128, NFLAT], BF16)
            dxn3v = dxn3[:].rearrange("p (r c) -> p r c", c=WP)
            # block 3 rows R=260..261 are streamed (zero-weighted) but never
            # written -> define once so no NaNs flow through the PE
            nc.gpsimd.memset(dxn3v[96:128, 260:262, :], 0.0)

            stats_sum = singles.tile([CH, NTILE], F32)
            stats_ssq = singles.tile([CH, NTILE], F32)

            for s in range(STEPS):
                xsrc = x_io if s == 0 else x_out

                # ---- per-step scalars ------------------------------------
                if s == 0:
                    tot_sum = scal[0:1, 0:1]
                    tot_ssq = scal[0:1, 1:2]
                else:
                    rsum = small.tile([CH, 1], F32)
                    nc.vector.tensor_reduce(rsum[:], stats_sum[:],
                                            axis=mybir.AxisListType.X, op=OP.add)
                    rssq = small.tile([CH, 1], F32)
                    nc.vector.tensor_reduce(rssq[:], stats_ssq[:],
                                            axis=mybir.AxisListType.X, op=OP.add)
                    arsum = small.tile([CH, 1], F32)
                    nc.gpsimd.partition_all_reduce(arsum[:], rsum[:], channels=CH,
                                                   reduce_op=bass_isa.ReduceOp.add)
                    arssq = small.tile([CH, 1], F32)
                    nc.gpsimd.partition_all_reduce(arssq[:], rssq[:], channels=CH,
                                                   reduce_op=bass_isa.ReduceOp.add)
                    tot_sum = small.tile([1, 1], F32)
                    nc.vector.tensor_add(tot_sum[:], arsum[0:1, 0:1], scal[0:1, 2:3])
                    tot_ssq = small.tile([1, 1], F32)
                    nc.vector.tensor_add(tot_ssq[:], arssq[0:1, 0:1], scal[0:1, 3:4])

                mu = sc.tile([1, 1], F32)
                nc.vector.tensor_scalar_mul(mu[:], tot_sum, 1.0 / NSTAT)
                ex2 = sc.tile([1, 1], F32)
                nc.vector.tensor_scalar_mul(ex2[:], tot_ssq, 1.0 / NSTAT)
                mu2 = sc.tile([1, 1], F32)
                nc.vector.tensor_mul(mu2[:], mu[:], mu[:])
                sd = sc.tile([1, 1], F32)
                nc.vector.tensor_tensor(out=sd[:], in0=ex2[:], in1=mu2[:],
                                        op=OP.subtract)
                nc.scalar.activation(sd[:], sd[:], AF.Sqrt, bias=eps_sb[:], scale=1.0)
                r11 = sc.tile([1, 1], F32)
                nc.vector.reciprocal(r11[:], sd[:])
                negmu = sc.tile([1, 1], F32)
                nc.vector.tensor_scalar_mul(negmu[:], mu[:], -1.0)
                nmur = sc.tile([1, 1], F32)
                nc.vector.tensor_mul(nmur[:], negmu[:], r11[:])

                r128 = sc.tile([128, 1], F32)
                nc.gpsimd.partition_broadcast(r128[:], r11[:], channels=128)
                nmur128 = sc.tile([128, 1], F32)
                nc.gpsimd.partition_broadcast(nmur128[:], nmur[:], channels=128)

                scale128 = sc.tile([128, 1], F32)
                nc.vector.tensor_scalar(out=scale128[:], in0=gb[:, 0:1],
                                        scalar1=r128[:, 0:1], scalar2=None,
                                        op0=OP.mult)
                bias128 = sc.tile([128, 1], F32)
                nc.vector.scalar_tensor_tensor(out=bias128[:], in0=gb[:, 0:1],
                                               scalar=nmur128[:, 0:1],
                                               in1=gb[:, 1:2],
                                               op0=OP.mult, op1=OP.add)
                t1 = sc.tile([HID, 1], F32)
                nc.vector.scalar_tensor_tensor(out=t1[:], in0=vecs[:, 1:2],
                                               scalar=r128[0:HID, 0:1],
                                               in1=vecs[:, 0:1],
                                               op0=OP.mult, op1=OP.add)
                bias_base = sc.tile([HID, 1], F32)
                nc.vector.scalar_tensor_tensor(out=bias_base[:], in0=vecs[:, 2:3],
                                               scalar=nmur128[0:HID, 0:1],
                                               in1=t1[:],
                                               op0=OP.mult, op1=OP.add)
                w2 = sc.tile([HID, 1], F32)
                nc.vector.tensor_scalar(out=w2[:], in0=vecs[:, 3:4],
                                        scalar1=r128[0:HID, 0:1], scalar2=None,
                                        op0=OP.mult)
                rampst = sc.tile([2, HID], BF16)
                nc.vector.tensor_scalar(out=rampst[:], in0=p12[:],
                                        scalar1=r128[0:2, 0:1], scalar2=None,
                                        op0=OP.mult)

                # ---- phase B: build dxn3 (block 1 = unshifted pad image) --
                # chunk partitions are (g, c): g = 4-row group, c = channel
                for rchunk in range(16):
                    ch16 = chunks.tile([128, 1024], F32)
                    for g in range(4):
                        nc.sync.dma_start(
                            ch16[32 * g:32 * g + 32, :],
                            xsrc[:, rchunk * 4096 + 1024 * g:
                                 rchunk * 4096 + 1024 * (g + 1)])
                    chbf = chunksb.tile([128, 1024], BF16)
                    nc.vector.tensor_scalar(out=chbf[:], in0=ch16[:],
                                            scalar1=scale128[:, 0:1],
                                            scalar2=bias128[:, 0:1],
                                            op0=OP.mult, op1=OP.add)
                    for g in range(4):
                        dst = dxn3v[32:64, 3 + 16 * rchunk + 4 * g:
                                    3 + 16 * rchunk + 4 * g + 4, 3:259]
                        nc.sync.dma_start(
                            dst, chbf[32 * g:32 * g + 32, :].rearrange(
                                "p (gr w) -> p gr w", w=256))

                # reflect halos on block 1: rows then cols
                for dst_r, src_r in ((2, 4), (1, 5), (0, 6),
                                     (259, 257), (260, 256), (261, 255)):
                    nc.sync.dma_start(dxn3v[32:64, dst_r:dst_r + 1, 3:259],
                                      dxn3v[32:64, src_r:src_r + 1, 3:259])
                for dst_c, src_c in ((2, 4), (1, 5), (0, 6),
                                     (259, 257), (260, 256), (261, 255)):
                    nc.vector.tensor_copy(dxn3v[32:64, :, dst_c:dst_c + 1],
                                          dxn3v[32:64, :, src_c:src_c + 1])

                # blocks 0,2,3 = flat-shifted copies of block 1
                nc.sync.dma_start(dxn3[0:32, WP:NFLAT], dxn3[32:64, 0:NFLAT - WP])
                nc.sync.dma_start(dxn3[64:96, 0:NFLAT - WP], dxn3[32:64, WP:NFLAT])
                nc.sync.dma_start(dxn3[96:128, 0:NFLAT - 2 * WP],
                                  dxn3[32:64, 2 * WP:NFLAT])

                # ---- phase C: 128 output tiles ---------------------------
                for p in range(NTILE):
                    h0 = 2 * p
                    ps1 = psA.tile([128, TPX], F32)
                    mm = 0
                    for rnd, dip in enumerate((-2, 2)):
                        for dj in range(-3, 4):
                            mov = dxn3v[:, h0 + 3 + dip:h0 + 5 + dip,
                                        3 + dj:259 + dj]
                            nc.tensor.matmul(
                                ps1[:], cstk[:, 128 * (7 * rnd + dj + 3):
                                             128 * (7 * rnd + dj + 4)],
                                mov, start=(mm == 0), stop=False)
                            mm += 1
                    nc.tensor.matmul(ps1[:], rampst[:, 0:HID], ramp[:],
                                     start=False, stop=True)

                    ps1v = ps1[:].rearrange("p (r c) -> p r c", c=256)
                    if p in FULL_TILES:
                        idx = FULL_TILES.index(p)
                        nc.vector.scalar_tensor_tensor(
                            out=ps1[:], in0=dcorr[:, TPX * idx:TPX * (idx + 1)],
                            scalar=r128[0:HID, 0:1], in1=ps1[:],
                            op0=OP.mult, op1=OP.add)
                    else:
                        off = 4 * TPX + 12 * (p - 2)
                        dl = dcorr[:, off:off + 6].rearrange("p (r c) -> p r c", c=3)
                        dr = dcorr[:, off + 6:off + 12].rearrange(
                            "p (r c) -> p r c", c=3)
                        nc.vector.scalar_tensor_tensor(
                            out=ps1v[:, :, 0:3], in0=dl, scalar=r128[0:HID, 0:1],
                            in1=ps1v[:, :, 0:3], op0=OP.mult, op1=OP.add)
                        nc.vector.scalar_tensor_tensor(
                            out=ps1v[:, :, 253:256], in0=dr,
                            scalar=r128[0:HID, 0:1],
                            in1=ps1v[:, :, 253:256], op0=OP.mult, op1=OP.add)

                    biasT = biasp.tile([HID, 1], F32)
                    nc.vector.scalar_tensor_tensor(out=biasT[:], in0=w2[:],
                                                   scalar=float(h0),
                                                   in1=bias_base[:],
                                                   op0=OP.mult, op1=OP.add)
                    # leaky_relu(z+b) = max(z+b, 0.01*(z+b)); bias-add on ACT
                    zb = hpool.tile([HID, TPX], F32, tag="zb")
                    nc.scalar.activation(zb[:], ps1[:], AF.Identity,
                                         bias=biasT[:, 0:1], scale=1.0)
                    hsb = hpool.tile([HID, TPX], BF16)
                    nc.vector.scalar_tensor_tensor(out=hsb[:], in0=zb[:],
                                                   scalar=0.01, in1=zb[:],
                                                   op0=OP.mult, op1=OP.max)

                    ps2 = psB.tile([CH, TPX], F32)
                    nc.tensor.matmul(ps2[:], fc1t[:], hsb[:], start=True, stop=True)

                    m32 = xio.tile([CH, TPX], BF16)
                    msl = mask_io[s:s + 1, TPX * p:TPX * (p + 1)]
                    mbc = bass.AP(tensor=msl.tensor, offset=msl.offset,
                                  ap=[[0, CH], [1, TPX]])
                    nc.sync.dma_start(m32[:], mbc)
                    xold = xio.tile([CH, TPX], F32)
                    nc.sync.dma_start(xold[:], xsrc[:, TPX * p:TPX * (p + 1)])

                    md = xio.tile([CH, TPX], F32)
                    nc.vector.tensor_mul(md[:], ps2[:], m32[:])
                    xnew = xio.tile([CH, TPX], F32)
                    if s == 0:
                        nc.vector.scalar_tensor_tensor(
                            out=xnew[:], in0=md[:], scalar=1.0, in1=xold[:],
                            op0=OP.bypass, op1=OP.add,
                            accum_out=stats_sum[:, p:p + 1])
                        junk = psJ.tile([CH, TPX], F32)
                        nc.scalar.activation(junk[:], xnew[:], AF.Square,
                                             accum_out=stats_ssq[:, p:p + 1])
                    else:
                        nc.vector.scalar_tensor_tensor(
                            out=xnew[:], in0=md[:], scalar=1.0, in1=xold[:],
                            op0=OP.bypass, op1=OP.add)
                    nc.sync.dma_start(x_out[:, TPX * p:TPX * (p + 1)], xnew[:])

    nc.compile()
    return nc


# ---------------------------------------------------------------------------
# host-side folding
# ---------------------------------------------------------------------------

def _fold_host(inputs):
    f64 = np.float64
    fc0_w = np.asarray(inputs["fc0_w"], f64)
    fc0_b = np.asarray(inputs["fc0_b"], f64)
    fc1_w = np.asarray(inputs["fc1_w"], f64)
    w1 = np.asarray(inputs["conv0_w"], f64)[:, 0].reshape(C, 49)
    w2 = np.asarray(inputs["conv1_w"], f64)[:, 0].reshape(C, 49)
    b1 = np.asarray(inputs["conv0_b"], f64)
    b2 = np.asarray(inputs["conv1_b"], f64)
    gamma = np.asarray(inputs["gn_gamma"], f64)
    beta = np.asarray(inputs["gn_beta"], f64)

    W_a, W_b, W_c = fc0_w[:, 0:C], fc0_w[:, C:2 * C], fc0_w[:, 2 * C:3 * C]
    Call = np.zeros((49, HID, C))
    for k in range(49):
        Call[k] = W_b * w1[None, :, k] + W_c * w2[None, :, k]
    Call[24] += W_a

    # stacked stationaries [128=(block,c), 14*128]: round 0 dip=-2, round 1 dip=+2
    cstk = np.zeros((128, 14 * 128), np.float32)
    for rnd, dip in enumerate((-2, 2)):
        for djj in range(7):
            col = 7 * rnd + djj
            for b in range(4):
                di = dip + (b - 1)
                if not -3 <= di <= 3:
                    continue
                k = (di + 3) * 7 + djj
                # lhsT[32b+c, hid] = C_k[hid, c]
                cstk[32 * b:32 * b + CH, 128 * col:128 * (col + 1)] = \
                    Call[k][:, 0:CH].T
    cstk = cstk.astype(ml_dtypes.bfloat16)

    # pos-channel fields (t-independent parts)
    pos_x = np.broadcast_to(np.linspace(1.0, 0.0, W)[None, :], (H, W))
    praw = np.stack([pos_x, pos_x.T])  # [2, H, W]
    praw_p = np.pad(praw, ((0, 0), (PAD, PAD), (PAD, PAD)), mode="reflect")
    Pg = np.zeros((HID, H, W))
    for k in range(49):
        di, dj = k // 7 - 3, k % 7 - 3
        sh = praw_p[:, PAD + di:PAD + di + H, PAD + dj:PAD + dj + W]
        Pg += gamma[CH] * Call[k][:, CH][:, None, None] * sh[0]
        Pg += gamma[CH + 1] * Call[k][:, CH + 1][:, None, None] * sh[1]
    Kc = Call.sum(0)[:, CH:C]                    # [128, 3]
    Kg = Kc @ gamma[CH:C]
    Kb = Kc @ beta[CH:C]
    K34 = Kc[:, 2] * gamma[CH + 2]               # alive-channel, times gamma

    p1 = Pg[:, 100, 101] - Pg[:, 100, 100]
    p2 = Pg[:, 101, 100] - Pg[:, 100, 100]
    p0_xy = Pg[:, 100, 100] - 100 * p1 - 100 * p2
    aff = (p0_xy[:, None, None]
           + p1[:, None, None] * np.arange(W)[None, None, :]
           + p2[:, None, None] * np.arange(H)[None, :, None])
    D = Pg - aff
    assert np.abs(D[:, PAD:H - PAD, PAD:W - PAD]).max() < 1e-9

    # D packed: 4 full tiles then 124 strips of (left [2,3], right [2,3])
    dpack = np.zeros((HID, 4 * TPX + 124 * 12), np.float32)
    for i, p in enumerate(FULL_TILES):
        dpack[:, TPX * i:TPX * (i + 1)] = D[:, 2 * p:2 * p + 2, :].reshape(HID, TPX)
    for p in range(2, 126):
        off = 4 * TPX + 12 * (p - 2)
        dpack[:, off:off + 6] = D[:, 2 * p:2 * p + 2, 0:3].reshape(HID, 6)
        dpack[:, off + 6:off + 12] = D[:, 2 * p:2 * p + 2, 253:256].reshape(HID, 6)

    convb_fold = W_b @ b1 + W_c @ b2
    bias_base = fc0_b + convb_fold + Kb

    ramp = np.zeros((2, TPX), np.float32)
    ramp[0] = np.tile(np.arange(256, dtype=np.float32), 2)
    ramp[1, 256:] = 1.0

    shared = dict(
        cstk=cstk,
        fc1t=np.asarray(inputs["fc1_w"], np.float32).T.astype(ml_dtypes.bfloat16),
        ramp=ramp.astype(ml_dtypes.bfloat16),
        p12=np.stack([p1, p2]).astype(np.float32),
        dcorr=dpack.astype(ml_dtypes.bfloat16),
        bias_base=bias_base.astype(np.float32),
        p0_xy=p0_xy.astype(np.float32),
        Kg=Kg.astype(np.float32),
        K34=K34.astype(np.float32),
        p2=p2.astype(np.float32),
        gamma=gamma.astype(np.float32),
        beta=beta.astype(np.float32),
        pos_xy_sum=float(praw.sum()),
        pos_xy_ssq=float((praw ** 2).sum()),
    )
    return shared


_NC_CACHE = {}


def kernel(**inputs):
    if "nc" not in _NC_CACHE:
        _NC_CACHE["nc"] = _build_nc()
    nc = _NC_CACHE["nc"]

    x = np.asarray(inputs["x"], np.float32)          # [8, 32, 256, 256]
    t = np.asarray(inputs["t"], np.float32)          # [8]
    rand_mask = np.asarray(inputs["rand_mask"], np.float32)  # [2, 8, W, H, 1]
    sh = _fold_host(inputs)

    # chunk partitions are (g, c): per-partition gamma/beta = tile-by-4
    gexp = np.tile(sh["gamma"][0:CH], 4)
    bexp = np.tile(sh["beta"][0:CH], 4)
    gb = np.stack([gexp, bexp], axis=1).astype(np.float32)   # [128, 2]

    in_maps = []
    for b in range(B):
        xb = x[b].reshape(CH, NPIX)
        mask = (np.transpose(rand_mask[:, b, :, :, 0], (0, 2, 1)) > FIRE)
        mask = mask.reshape(STEPS, NPIX).astype(ml_dtypes.bfloat16)
        tb = float(t[b])

        pos_sum = sh["pos_xy_sum"] + tb * NPIX
        pos_ssq = sh["pos_xy_ssq"] + tb * tb * NPIX
        sum0 = float(xb.astype(np.float64).sum()) + pos_sum
        ssq0 = float((xb.astype(np.float64) ** 2).sum()) + pos_ssq

        vecs = np.stack([
            sh["bias_base"],
            sh["p0_xy"] + tb * sh["K34"],
            sh["Kg"],
            sh["p2"],
        ], axis=1).astype(np.float32)                 # [128, 4]

        in_maps.append({
            "x_io": np.ascontiguousarray(xb),
            "cstk_io": sh["cstk"],
            "fc1t_io": sh["fc1t"],
            "ramp_io": sh["ramp"],
            "p12_io": sh["p12"],
            "vecs_io": vecs,
            "gb_io": gb,
            "dcorr_io": sh["dcorr"],
            "mask_io": mask,
            "scal_io": np.array([[sum0, ssq0, pos_sum, pos_ssq]], np.float32),
        })

    res = run_bass_kernel_spmd(nc, in_maps, core_ids=list(range(N_CORES)))
    _NC_CACHE["last_results"] = res
    out = np.stack([res.results[b]["x_out"].reshape(CH, H, W) for b in range(B)])
    return out.astype(np.float32)
